# revision 1
# baseline (speedup 1.0000x reference)
"""Trainium2 Bass kernel for nn_ContrastiveLearning (NT-Xent over pairwise
symmetrized-KL of diagonal Gaussians).

Math (equivalent reformulation of the reference):
  loss[i,j] = -0.25*(A[i,j] + A[j,i] + md[i,j] + md[j,i] - 2D)   (ld terms cancel)
  A[i,j]+A[j,i] + md[i,j]+md[j,i] - (q_i + q_j)
      = <u_i,s_j> + <s_i,u_j> - 2<w_i,m_j> - 2<m_i,w_j>  =: total[i,j]
  with u=1/sigma, s=sigma+mu^2, w=mu/sigma, m=mu, q_x=<mu^2,1/sigma>_x.
  Row-constant factors cancel in lf_i = -log2(num_i)+log2(den_i)+log2(cnt_i),
  so the device computes E[i,j] = exp(-0.25*(total[i,j] + q_j + pen_j) + C)
  where pen_j = BIG kills padding columns, the diagonal is killed by adding
  BIG*I to total via an extra matmul, and C keeps fp32 exp in range.
  num_i = sum_j E*[lab_i==lab_j],  den_i = sum_j E.
  cnt, log2 and the final masked mean are O(N) host work.

Sharding: anchor rows are data-parallel over 8 cores. Invalid tokens
(mask==0) are compacted out on the host (they contribute nothing), the
remainder padded to a multiple of 1024. Each core receives the token axis
rotated by core*N/8 so its anchors are always tokens [0, N/8) — a single
SPMD program with static addressing.
"""
import numpy as np

EMBED_DIM = 128
H_DIM = 768
C_SHIFT = 40.0           # global exponent shift, cancels between num and den
BIG = 1024.0 * 1024.0    # diag/pad kill: exp(-0.25*BIG) == 0 in fp32
SQRT_BIG = 1024.0

_cache = {}


def _build(n_pad):
    import concourse.bass as bass
    import concourse.tile as tile
    from concourse import bacc, mybir, masks

    f32 = mybir.dt.float32
    f32r = mybir.dt.float32r
    AF = mybir.ActivationFunctionType
    ALU = mybir.AluOpType
    AX = mybir.AxisListType

    n_groups = n_pad // 512          # 512-token groups
    n_anchor = n_pad // 8            # anchors per core
    n_itiles = n_anchor // 128       # I-tiles per core
    n_jtiles = n_pad // 512          # J-tiles (512 wide)

    nc = bacc.Bacc(None, target_bir_lowering=False, debug=False)
    emb_d = nc.declare_dram_parameter("emb", [n_pad, H_DIM], f32, isOutput=False)
    wmu_d = nc.declare_dram_parameter("wmu", [H_DIM, EMBED_DIM], f32, isOutput=False)
    wsig_d = nc.declare_dram_parameter("wsig", [H_DIM, EMBED_DIM], f32, isOutput=False)
    bmu_d = nc.declare_dram_parameter("bmu", [EMBED_DIM], f32, isOutput=False)
    bsig_d = nc.declare_dram_parameter("bsig", [EMBED_DIM], f32, isOutput=False)
    labc_d = nc.declare_dram_parameter("labc", [n_pad], f32, isOutput=False)
    pen_d = nc.declare_dram_parameter("pen", [n_pad], f32, isOutput=False)
    out_d = nc.declare_dram_parameter("out", [n_anchor, 2], f32, isOutput=True)

    with tile.TileContext(nc) as tc:
        with tc.tile_pool(name="const", bufs=1) as const, \
             tc.tile_pool(name="feat", bufs=1) as feat, \
             tc.tile_pool(name="chunk", bufs=3) as chunk, \
             tc.tile_pool(name="grp", bufs=1) as grp, \
             tc.tile_pool(name="work", bufs=2) as work, \
             tc.tile_pool(name="acc", bufs=1) as accp, \
             tc.tile_pool(name="psA", bufs=2, space="PSUM") as psA, \
             tc.tile_pool(name="psB", bufs=1, space="PSUM") as psB, \
             tc.tile_pool(name="psC", bufs=2, space="PSUM") as psC:

            # ---------------- constants / small loads ----------------
            wmu_t = const.tile([128, 6, 128], f32r)
            wsig_t = const.tile([128, 6, 128], f32r)
            nc.sync.dma_start(wmu_t[:], wmu_d.rearrange("(a b) c -> b a c", b=128).bitcast(f32r))
            nc.sync.dma_start(wsig_t[:], wsig_d.rearrange("(a b) c -> b a c", b=128).bitcast(f32r))
            bmu_t = const.tile([128, 1], f32)
            bsig_t = const.tile([128, 1], f32)
            nc.sync.dma_start(bmu_t[:], bmu_d.rearrange("(p o) -> p o", o=1))
            nc.sync.dma_start(bsig_t[:], bsig_d.rearrange("(p o) -> p o", o=1))
            labcb_t = const.tile([128, n_pad], f32)
            nc.sync.dma_start(labcb_t[:], labc_d[:].partition_broadcast(128))
            pen_t = const.tile([1, n_pad], f32r)
            nc.sync.dma_start(pen_t[:], pen_d.rearrange("(o n) -> o n", o=1).bitcast(f32r))
            labr_t = const.tile([128, n_itiles], f32)
            nc.sync.dma_start(
                labr_t[:],
                labc_d[0:n_anchor].rearrange("(t p) -> p t", p=128),
            )

            identity0 = const.tile([128, 128], f32)
            masks.make_identity(nc, identity0[:])
            eqd_r = const.tile([128, 128], f32r)
            nc.vector.tensor_scalar(eqd_r[:], identity0[:], SQRT_BIG, None, ALU.mult)
            onescol0 = const.tile([128, 1], f32)
            nc.vector.memset(onescol0[:], 1.0)
            onescol_r = const.tile([128, 1], f32r)
            nc.vector.tensor_copy(onescol_r[:], onescol0[:])
            onesrow0 = const.tile([1, 128], f32)
            nc.vector.memset(onesrow0[:], 1.0)
            onesrow_r = const.tile([1, 128], f32r)
            nc.vector.tensor_copy(onesrow_r[:], onesrow0[:])
            ones1_r = const.tile([1, 1], f32r)
            nc.vector.memset(ones1_r[:].bitcast(f32), 1.0)
            cbias_t = const.tile([128, 1], f32)
            nc.vector.memset(cbias_t[:], C_SHIFT)

            # ---------------- persistent feature tensors ----------------
            u_f = feat.tile([128, n_pad], f32r)      # 1/sigma          (G_col)
            s_f = feat.tile([128, n_pad], f32r)      # sigma + mu^2     (G_col)
            m2_f = feat.tile([128, n_pad], f32r)     # -2*mu            (G_col)
            w2_f = feat.tile([128, n_pad], f32r)     # -2*mu/sigma      (G_col)
            qcol = feat.tile([1, n_pad], f32r)       # q_j + pen_j
            mu_a = feat.tile([128, n_anchor], f32r)  # mu       (G_row anchors)
            w_a = feat.tile([128, n_anchor], f32r)   # mu/sigma (G_row anchors)

            # ---------------- phase 1 ----------------
            for g in range(n_groups):
                gs = slice(g * 512, (g + 1) * 512)
                h_ts = []
                for c in range(4):
                    tok0 = g * 512 + c * 128
                    e_t = chunk.tile([128, H_DIM], f32, tag="emb")
                    nc.sync.dma_start(e_t[:], emb_d[tok0:tok0 + 128, :])
                    h_t = chunk.tile([128, H_DIM], f32, tag="h", bufs=5)
                    nc.gpsimd.tensor_scalar_max(h_t[:], e_t[:], 0.0)
                    h_ts.append(h_t)
                hT = []
                for kk in range(6):
                    ps_tr = psA.tile([128, 512], f32, tag="tr")
                    for c in range(4):
                        nc.tensor.transpose(
                            ps_tr[:, c * 128:(c + 1) * 128],
                            h_ts[c][:, kk * 128:(kk + 1) * 128],
                            identity0[:],
                        )
                    hT_k = grp.tile([128, 512], f32r, tag=f"hT{kk}")
                    if kk % 3 == 0:
                        nc.scalar.copy(hT_k[:], ps_tr[:])
                    else:
                        nc.vector.tensor_copy(hT_k[:], ps_tr[:])
                    hT.append(hT_k)

                ps_mu = psB.tile([128, 512], f32, tag="mu")
                ps_z = psB.tile([128, 512], f32, tag="z")
                for kk in range(6):
                    nc.tensor.matmul(ps_mu[:], wmu_t[:, kk, :], hT[kk][:],
                                     start=(kk == 0), stop=(kk == 5))
                for kk in range(6):
                    nc.tensor.matmul(ps_z[:], wsig_t[:, kk, :], hT[kk][:],
                                     start=(kk == 0), stop=(kk == 5))

                mu_g = work.tile([128, 512], f32, tag="mu_g")
                nc.scalar.activation(mu_g[:], ps_mu[:], AF.Identity, bias=bmu_t[:])
                zsb = work.tile([128, 512], f32, tag="zsb")
                nc.scalar.activation(zsb[:], ps_z[:], AF.Identity, bias=bsig_t[:])
                zm = work.tile([128, 512], f32, tag="zm")
                nc.gpsimd.tensor_scalar_min(zm[:], zsb[:], 0.0)
                e1 = work.tile([128, 512], f32, tag="e1")
                nc.scalar.activation(e1[:], zm[:], AF.Exp)
                zp = work.tile([128, 512], f32, tag="zp")
                nc.scalar.activation(zp[:], zsb[:], AF.Relu)
                # sigma = (e1 + 1e-14) + zp
                sig_g = work.tile([128, 512], f32, tag="sig_g")
                nc.vector.scalar_tensor_tensor(sig_g[:], e1[:], 1e-14, zp[:],
                                               ALU.add, ALU.add)
                ls = work.tile([128, 512], f32, tag="ls")
                nc.scalar.activation(ls[:], sig_g[:], AF.Ln)
                nc.scalar.activation(u_f[:, gs], ls[:], AF.Exp, scale=-1.0)
                psq = work.tile([128, 512], f32, tag="psq")
                nc.scalar.activation(psq[:], mu_g[:], AF.Square)
                nc.vector.tensor_add(s_f[:, gs], psq[:], sig_g[:])
                nc.gpsimd.tensor_scalar_mul(m2_f[:, gs], mu_g[:], -2.0)
                nc.vector.scalar_tensor_tensor(w2_f[:, gs], mu_g[:], -2.0,
                                               u_f[:, gs].bitcast(f32),
                                               ALU.mult, ALU.mult)
                pu_g = work.tile([128, 512], f32r, tag="pu_g")
                nc.vector.tensor_mul(pu_g[:], psq[:],
                                     u_f[:, gs].bitcast(f32))
                if g == 0:
                    nc.vector.tensor_copy(mu_a[:], mu_g[:, 0:n_anchor])
                    nc.vector.scalar_tensor_tensor(w_a[:],
                                                   mu_g[:, 0:n_anchor], 1.0,
                                                   u_f[:, 0:n_anchor].bitcast(f32),
                                                   ALU.mult, ALU.mult)
                # q row: ones^T @ pu + 1^T @ pen
                ps_q = psC.tile([1, 512], f32, tag="q")
                nc.tensor.matmul(ps_q[:], onescol_r[:], pu_g[:],
                                 start=True, stop=False)
                nc.tensor.matmul(ps_q[:], ones1_r[:], pen_t[:, gs],
                                 start=False, stop=True)
                nc.scalar.copy(qcol[:, gs], ps_q[:])

            # ---------------- phase 2 ----------------
            num_sl = accp.tile([128, n_itiles, n_jtiles], f32)
            den_sl = accp.tile([128, n_itiles, n_jtiles], f32)
            for t in range(n_itiles):
                isl = slice(t * 128, (t + 1) * 128)
                dj = (t * 128) // 512        # J-tile containing the diagonal
                doff = (t * 128) % 512
                for jt in range(n_jtiles):
                    jsl = slice(jt * 512, (jt + 1) * 512)
                    ps2 = psC.tile([128, 512], f32, tag="p2")
                    nc.tensor.matmul(ps2[:], s_f[:, isl], u_f[:, jsl],
                                     start=True, stop=False)
                    nc.tensor.matmul(ps2[:], u_f[:, isl], s_f[:, jsl],
                                     start=False, stop=False)
                    nc.tensor.matmul(ps2[:], w_a[:, isl], m2_f[:, jsl],
                                     start=False, stop=False)
                    nc.tensor.matmul(ps2[:], mu_a[:, isl], w2_f[:, jsl],
                                     start=False, stop=False)
                    last = (jt != dj)
                    nc.tensor.matmul(ps2[:], onesrow_r[:], qcol[:, jsl],
                                     start=False, stop=last)
                    if jt == dj:
                        nc.tensor.matmul(ps2[:, doff:doff + 128], eqd_r[:], eqd_r[:],
                                         start=False, stop=True)
                    e_t = work.tile([128, 512], f32, tag="E")
                    nc.scalar.activation(e_t[:], ps2[:], AF.Exp, scale=-0.25,
                                         bias=cbias_t[:],
                                         accum_out=den_sl[:, t, jt:jt + 1])
                    labq = work.tile([128, 512], f32, tag="labq")
                    nc.gpsimd.tensor_scalar(labq[:], labcb_t[:, jsl],
                                            labr_t[:, t:t + 1], None, ALU.is_equal)
                    msk = work.tile([128, 512], f32, tag="msk")
                    nc.vector.tensor_mul(msk[:], e_t[:], labq[:])
                    nc.vector.tensor_reduce(num_sl[:, t, jt:jt + 1], msk[:],
                                            AX.X, ALU.add)

            nd = accp.tile([128, n_itiles, 2], f32)
            for t in range(n_itiles):
                nc.vector.tensor_reduce(nd[:, t, 0:1], num_sl[:, t, :], AX.X, ALU.add)
                nc.vector.tensor_reduce(nd[:, t, 1:2], den_sl[:, t, :], AX.X, ALU.add)
                nc.sync.dma_start(
                    out_d[t * 128:(t + 1) * 128, :], nd[:, t, :])

    nc.compile()
    return nc


def kernel(ent_embeddings, ent_type_ids, ent_mask, W_mu, b_mu, W_sigma, b_sigma):
    from concourse.bass_utils import run_bass_kernel_spmd

    emb = np.ascontiguousarray(np.asarray(ent_embeddings, dtype=np.float32)).reshape(-1, H_DIM)
    labels = np.asarray(ent_type_ids).reshape(-1).astype(np.int64)
    mask = np.asarray(ent_mask).reshape(-1).astype(np.int64)
    W_mu = np.ascontiguousarray(np.asarray(W_mu, dtype=np.float32))
    W_sigma = np.ascontiguousarray(np.asarray(W_sigma, dtype=np.float32))
    b_mu = np.ascontiguousarray(np.asarray(b_mu, dtype=np.float32))
    b_sigma = np.ascontiguousarray(np.asarray(b_sigma, dtype=np.float32))

    valid = (mask == 1) & (labels >= 0)
    vidx = np.nonzero(valid)[0]
    n_v = len(vidx)
    if n_v == 0:
        return np.float32(0.0)

    n_pad = 1024 if n_v <= 1024 else 2048
    assert n_v <= n_pad, f"too many valid tokens: {n_v}"
    n_anchor = n_pad // 8

    emb_v = np.zeros((n_pad, H_DIM), dtype=np.float32)
    emb_v[:n_v] = emb[vidx]
    labc_v = np.full(n_pad, -1.0, dtype=np.float32)
    labc_v[:n_v] = labels[vidx].astype(np.float32)
    pen_v = np.full(n_pad, BIG, dtype=np.float32)
    pen_v[:n_v] = 0.0

    if n_pad not in _cache:
        _cache[n_pad] = _build(n_pad)
    nc = _cache[n_pad]

    in_maps = []
    for c in range(8):
        r = c * n_anchor
        in_maps.append({
            "emb": np.roll(emb_v, -r, axis=0),
            "wmu": W_mu, "wsig": W_sigma, "bmu": b_mu, "bsig": b_sigma,
            "labc": np.roll(labc_v, -r),
            "pen": np.roll(pen_v, -r),
        })

    res = run_bass_kernel_spmd(nc, in_maps, list(range(8)))

    num = np.empty(n_pad, dtype=np.float32)
    den = np.empty(n_pad, dtype=np.float32)
    for c in range(8):
        nd = res.results[c]["out"]
        rows = (np.arange(n_anchor) + c * n_anchor) % n_pad
        num[rows] = nd[:, 0]
        den[rows] = nd[:, 1]

    # host-side epilogue on the n_v real rows
    labs = labels[vidx]
    hist = np.bincount(labs, minlength=int(labs.max()) + 1)
    cnt = (hist[labs] - 1).astype(np.float64)
    sel = cnt > 0
    n_sel = max(sel.sum(), 1)
    num_v = num[:n_v].astype(np.float64)
    den_v = den[:n_v].astype(np.float64)
    safe_num = np.where(sel, num_v, 1.0)
    safe_den = np.where(sel, den_v, 1.0)
    safe_cnt = np.where(sel, cnt, 1.0)
    lf = (np.log(safe_den) - np.log(safe_num)) / np.log(2.0) + np.log2(safe_cnt)
    total = np.sum(np.where(sel, lf, 0.0)) / n_sel
    return np.float32(total)



# revision 11
# speedup vs baseline: 4.3790x; 4.3790x over previous
"""Trainium2 Bass kernel for nn_ContrastiveLearning (NT-Xent over pairwise
symmetrized-KL of diagonal Gaussians).

Math (equivalent reformulation of the reference):
  loss[i,j] = -0.25*(A[i,j] + A[j,i] + md[i,j] + md[j,i] - 2D)   (ld terms cancel)
  total[i,j] := <s_i,u_j> + <u_i,s_j> - 2<w_i,m_j> - 2<m_i,w_j>
  with u=1/sigma, s=sigma+mu^2, w=mu/sigma, m=mu.
  Row-constant factors cancel in lf_i = -log2(num_i)+log2(den_i)+log2(cnt_i),
  so the device computes E[i,j] = exp(-0.25*(total[i,j] + q_j + pen_j) + C)
  where q_j = <mu_j^2, 1/sigma_j>, pen_j = BIG kills padding columns, the
  diagonal is killed by adding BIG*I via an extra matmul, and C keeps fp32
  exp in range.  num_i = sum_j E*eq[i,j], den_i = sum_j E.
  cnt, log2 and the final masked mean are O(N) host work.

Sharding: anchor rows are data-parallel over 8 cores. Invalid tokens are
compacted out on the host, the remainder padded to a multiple of 1024. Each
core receives the token axis rotated by core*N/8 so its anchors are always
tokens [0, n_anchor) — a single SPMD program with static addressing.

Device dtype strategy (validated against fp64 reference in simulation,
rel err ~1e-5 at the final scalar): h, W, and the pairwise features
(mu, u, s, w) are bf16 (halves DMA, 2x PE); the sigma chain, q column
sums, exponents and num/den accumulations stay fp32.
"""
import numpy as np

EMBED_DIM = 128
H_DIM = 768
C_SHIFT = 40.0           # global exponent shift, cancels between num and den
BIG = 1024.0 * 1024.0    # diag/pad kill: exp(-0.25*BIG) == 0 in fp32
SQRT_BIG = 1024.0

_cache = {}


def _build(n_pad):
    import concourse.bass as bass
    import concourse.tile as tile
    from concourse import bacc, mybir

    f32 = mybir.dt.float32
    f32r = mybir.dt.float32r
    bf16 = mybir.dt.bfloat16
    AF = mybir.ActivationFunctionType
    ALU = mybir.AluOpType
    AX = mybir.AxisListType

    n_groups = n_pad // 512          # 512-token groups
    n_anchor = n_pad // 8            # anchors per core
    n_itiles = n_anchor // 128       # I-tiles per core
    n_jtiles = n_pad // 512          # J-tiles (512 wide)

    nc = bacc.Bacc(None, target_bir_lowering=False, debug=False)
    # h^T = emb^T (pre-transposed on host, bf16); relu applied on device
    ht_d = nc.declare_dram_parameter("ht", [H_DIM, n_pad], bf16, isOutput=False)
    wmu_d = nc.declare_dram_parameter("wmu", [H_DIM, EMBED_DIM], bf16, isOutput=False)
    wsig_d = nc.declare_dram_parameter("wsig", [H_DIM, EMBED_DIM], bf16, isOutput=False)
    bmu_d = nc.declare_dram_parameter("bmu", [EMBED_DIM], f32, isOutput=False)
    bsig_d = nc.declare_dram_parameter("bsig", [EMBED_DIM], f32, isOutput=False)
    eq_d = nc.declare_dram_parameter("eq", [n_anchor, n_pad], f32, isOutput=False)
    pen_d = nc.declare_dram_parameter("pen", [n_pad], f32, isOutput=False)
    eqd_d = nc.declare_dram_parameter("eqd", [128, 128], bf16, isOutput=False)
    out_d = nc.declare_dram_parameter("out", [n_anchor, 2], f32, isOutput=True)

    with tile.TileContext(nc) as tc:
        with tc.tile_pool(name="const", bufs=1) as const, \
             tc.tile_pool(name="feat", bufs=1) as feat, \
             tc.tile_pool(name="chunk", bufs=2) as chunk, \
             tc.tile_pool(name="work", bufs=2) as work, \
             tc.tile_pool(name="acc", bufs=1) as accp, \
             tc.tile_pool(name="psP", bufs=2, space="PSUM") as psP, \
             tc.tile_pool(name="psQ", bufs=2, space="PSUM") as psQ, \
             tc.tile_pool(name="ps2", bufs=2, space="PSUM") as ps2p:

            # ---- prime activation tables on a dummy tile (overlaps DMAs) ----
            prime = const.tile([128, 1], f32)
            nc.vector.memset(prime[:], 1.0)
            pout = const.tile([128, 1], f32)
            nc.scalar.activation(pout[:], prime[:], AF.Identity)
            nc.scalar.activation(pout[:], prime[:], AF.Relu)
            nc.scalar.activation(pout[:], prime[:], AF.Exp)
            nc.scalar.activation(pout[:], prime[:], AF.Exp, scale=-1.0)
            nc.scalar.activation(pout[:], prime[:], AF.Ln)
            nc.scalar.activation(pout[:], prime[:], AF.Exp, scale=-0.25)
            nc.scalar.activation(pout[:], prime[:], AF.Copy)

            # ---------------- constants / small loads ----------------
            wmu_t = const.tile([128, 6, 128], bf16)
            wsig_t = const.tile([128, 6, 128], bf16)
            nc.sync.dma_start(wmu_t[:], wmu_d.rearrange("(a b) c -> b a c", b=128))
            nc.sync.dma_start(wsig_t[:], wsig_d.rearrange("(a b) c -> b a c", b=128))
            bmu_t = const.tile([128, 1], f32)
            bsig_t = const.tile([128, 1], f32)
            nc.sync.dma_start(bmu_t[:], bmu_d.rearrange("(p o) -> p o", o=1))
            nc.sync.dma_start(bsig_t[:], bsig_d.rearrange("(p o) -> p o", o=1))
            eq_t = const.tile([128, n_itiles, n_pad], f32)
            nc.sync.dma_start(eq_t[:], eq_d.rearrange("(t p) n -> p t n", p=128))
            pen_t = const.tile([1, n_pad], f32r)
            nc.sync.dma_start(pen_t[:], pen_d.rearrange("(o n) -> o n", o=1).bitcast(f32r))

            eqd_t = const.tile([128, 128], bf16)
            nc.sync.dma_start(eqd_t[:], eqd_d[:, :])
            ones0 = const.tile([128, 1], f32)
            nc.vector.memset(ones0[:], 1.0)
            onescol_b = const.tile([128, 1], f32r)
            nc.vector.tensor_copy(onescol_b[:], ones0[:])
            ones1_r = const.tile([1, 1], f32r)
            nc.vector.tensor_copy(ones1_r[:], ones0[0:1, :])
            onesrow0 = const.tile([1, 128], f32)
            nc.vector.memset(onesrow0[:], 1.0)
            onesrow_b = const.tile([1, 128], f32r)
            nc.vector.tensor_copy(onesrow_b[:], onesrow0[:])
            cbias_t = const.tile([128, 1], f32)
            nc.vector.memset(cbias_t[:], C_SHIFT)

            # ---------------- persistent feature tensors (bf16) ----------------
            mu_f = feat.tile([128, n_pad], bf16)     # mu
            u_f = feat.tile([128, n_pad], bf16)      # 1/sigma
            s_f = feat.tile([128, n_pad], bf16)      # sigma + mu^2
            wc_f = feat.tile([128, n_pad], bf16)     # mu/sigma
            qcol = feat.tile([1, n_pad], f32r)       # q_j + pen_j

            ht_r = ht_d.rearrange("(a b) c -> b a c", b=128)  # [128, 6, n_pad]

            # ---------------- phase 1 ----------------
            for g in range(n_groups):
                gs = slice(g * 512, (g + 1) * 512)
                hg = []
                for kk in range(6):
                    hraw = chunk.tile([128, 512], bf16, tag=f"hraw{kk}")
                    nc.sync.dma_start(hraw[:], ht_r[:, kk, gs])
                    hk = chunk.tile([128, 512], bf16, tag=f"hg{kk}")
                    if kk % 2 == 0:
                        nc.vector.tensor_scalar_max(hk[:], hraw[:], 0.0)
                    else:
                        nc.scalar.activation(hk[:], hraw[:], AF.Relu)
                    hg.append(hk)

                ps_mu = psP.tile([128, 512], f32, tag="mu")
                ps_z = psP.tile([128, 512], f32, tag="z")
                for kk in range(6):
                    nc.tensor.matmul(ps_mu[:], wmu_t[:, kk, :], hg[kk][:],
                                     start=(kk == 0), stop=(kk == 5))
                for kk in range(6):
                    nc.tensor.matmul(ps_z[:], wsig_t[:, kk, :], hg[kk][:],
                                     start=(kk == 0), stop=(kk == 5))

                # mu (bf16 feature) straight out of PSUM with bias
                nc.scalar.activation(mu_f[:, gs], ps_mu[:], AF.Identity, bias=bmu_t[:])
                # sigma = exp(min(z,0)) + 1e-14 + relu(z)   (elu(z)+1)
                zm = work.tile([128, 512], f32, tag="zm")
                nc.vector.tensor_scalar(zm[:], ps_z[:], bsig_t[:], 0.0, ALU.add, ALU.min)
                zp = work.tile([128, 512], f32, tag="zp")
                nc.vector.tensor_scalar(zp[:], ps_z[:], bsig_t[:], 0.0, ALU.add, ALU.max)
                e1 = work.tile([128, 512], f32, tag="e1")
                nc.scalar.activation(e1[:], zm[:], AF.Exp)
                sig = work.tile([128, 512], f32, tag="sig")
                nc.vector.scalar_tensor_tensor(sig[:], e1[:], 1e-14, zp[:],
                                               ALU.add, ALU.add)
                ls = work.tile([128, 512], f32, tag="ls")
                nc.scalar.activation(ls[:], sig[:], AF.Ln)
                nc.scalar.activation(u_f[:, gs], ls[:], AF.Exp, scale=-1.0)
                psq = work.tile([128, 512], f32, tag="psq")
                nc.vector.tensor_mul(psq[:], mu_f[:, gs], mu_f[:, gs])
                nc.vector.tensor_add(s_f[:, gs], psq[:], sig[:])
                nc.vector.tensor_mul(wc_f[:, gs], mu_f[:, gs], u_f[:, gs])
                pu = work.tile([128, 512], f32r, tag="pu")
                nc.vector.tensor_mul(pu[:], mu_f[:, gs], wc_f[:, gs])
                # q row: ones^T @ pu + 1^T @ pen
                ps_q = psQ.tile([1, 512], f32, tag="q")
                nc.tensor.matmul(ps_q[:], onescol_b[:], pu[:],
                                 start=True, stop=False)
                nc.tensor.matmul(ps_q[:], ones1_r[:], pen_t[:, gs],
                                 start=False, stop=True)
                nc.scalar.copy(qcol[:, gs], ps_q[:])

            # anchor-side stationaries carry the -2 factor of the cross terms
            m2a = feat.tile([128, n_anchor], bf16)
            nc.vector.tensor_scalar_mul(m2a[:], mu_f[:, 0:n_anchor], -2.0)
            w2a = feat.tile([128, n_anchor], bf16)
            nc.vector.tensor_scalar_mul(w2a[:], wc_f[:, 0:n_anchor], -2.0)

            # ---------------- phase 2 ----------------
            nt = n_itiles * n_jtiles
            num_sl = accp.tile([128, nt], f32)
            den_sl = accp.tile([128, nt], f32)
            for t in range(n_itiles):
                isl = slice(t * 128, (t + 1) * 128)
                dj = (t * 128) // 512        # J-tile containing the diagonal
                doff = (t * 128) % 512
                for jt in range(n_jtiles):
                    jsl = slice(jt * 512, (jt + 1) * 512)
                    ix = t * n_jtiles + jt
                    ps2 = ps2p.tile([128, 512], f32, tag="p2")
                    nc.tensor.matmul(ps2[:], s_f[:, isl], u_f[:, jsl],
                                     start=True, stop=False)
                    nc.tensor.matmul(ps2[:], u_f[:, isl], s_f[:, jsl],
                                     start=False, stop=False)
                    nc.tensor.matmul(ps2[:], w2a[:, isl], mu_f[:, jsl],
                                     start=False, stop=False)
                    nc.tensor.matmul(ps2[:], m2a[:, isl], wc_f[:, jsl],
                                     start=False, stop=False)
                    last = (jt != dj)
                    nc.tensor.matmul(ps2[:], onesrow_b[:], qcol[:, jsl],
                                     start=False, stop=last)
                    if jt == dj:
                        nc.tensor.matmul(ps2[:, doff:doff + 128], eqd_t[:], eqd_t[:],
                                         start=False, stop=True)
                    e_t = work.tile([128, 512], f32, tag="E")
                    nc.scalar.activation(e_t[:], ps2[:], AF.Exp, scale=-0.25,
                                         bias=cbias_t[:],
                                         accum_out=den_sl[:, ix:ix + 1])
                    msk = work.tile([128, 512], f32, tag="msk")
                    nc.vector.scalar_tensor_tensor(msk[:], e_t[:], 1.0,
                                                   eq_t[:, t, jsl],
                                                   ALU.mult, ALU.mult,
                                                   accum_out=num_sl[:, ix:ix + 1])

            nd = accp.tile([128, n_itiles, 2], f32)
            for t in range(n_itiles):
                nc.vector.tensor_reduce(nd[:, t, 0:1],
                                        num_sl[:, t * n_jtiles:(t + 1) * n_jtiles],
                                        AX.X, ALU.add)
                nc.vector.tensor_reduce(nd[:, t, 1:2],
                                        den_sl[:, t * n_jtiles:(t + 1) * n_jtiles],
                                        AX.X, ALU.add)
                nc.sync.dma_start(
                    out_d[t * 128:(t + 1) * 128, :], nd[:, t, :])

    nc.compile()
    return nc


def _prep(ent_embeddings, ent_type_ids, ent_mask, W_mu, b_mu, W_sigma, b_sigma):
    """Host-side compaction / layout. Returns (in_maps, meta) or (None, scalar)."""
    import ml_dtypes
    bf = ml_dtypes.bfloat16

    emb = np.ascontiguousarray(np.asarray(ent_embeddings, dtype=np.float32)).reshape(-1, H_DIM)
    labels = np.asarray(ent_type_ids).reshape(-1).astype(np.int64)
    mask = np.asarray(ent_mask).reshape(-1).astype(np.int64)
    W_mu = np.asarray(W_mu, dtype=np.float32)
    W_sigma = np.asarray(W_sigma, dtype=np.float32)
    b_mu = np.ascontiguousarray(np.asarray(b_mu, dtype=np.float32))
    b_sigma = np.ascontiguousarray(np.asarray(b_sigma, dtype=np.float32))

    valid = (mask == 1) & (labels >= 0)
    vidx = np.nonzero(valid)[0]
    n_v = len(vidx)
    if n_v == 0:
        return None, np.float32(0.0)

    n_pad = 1024 if n_v <= 1024 else 2048
    assert n_v <= n_pad, f"too many valid tokens: {n_v}"
    n_anchor = n_pad // 8

    embT = np.zeros((H_DIM, n_pad), dtype=bf)
    embT[:, :n_v] = emb[vidx].T.astype(bf)
    labc_v = np.full(n_pad, -1.0, dtype=np.float32)
    labc_v[:n_v] = labels[vidx].astype(np.float32)
    pen_v = np.full(n_pad, BIG, dtype=np.float32)
    pen_v[:n_v] = 0.0
    wmu_b = np.ascontiguousarray(W_mu.astype(bf))
    wsig_b = np.ascontiguousarray(W_sigma.astype(bf))
    eqd = (np.eye(128, dtype=np.float32) * SQRT_BIG).astype(bf)

    in_maps = []
    for c in range(8):
        r = c * n_anchor
        labr = np.roll(labc_v, -r)
        eq = (labr[:n_anchor, None] == labr[None, :]).astype(np.float32)
        in_maps.append({
            "ht": np.ascontiguousarray(np.roll(embT, -r, axis=1)),
            "wmu": wmu_b, "wsig": wsig_b, "bmu": b_mu, "bsig": b_sigma,
            "eq": eq,
            "pen": np.roll(pen_v, -r),
            "eqd": eqd,
        })
    meta = (labels, vidx, n_v, n_pad, n_anchor)
    return in_maps, meta


def _epilogue(res, meta):
    labels, vidx, n_v, n_pad, n_anchor = meta
    num = np.empty(n_pad, dtype=np.float32)
    den = np.empty(n_pad, dtype=np.float32)
    for c in range(8):
        nd = res.results[c]["out"]
        rows = (np.arange(n_anchor) + c * n_anchor) % n_pad
        num[rows] = nd[:, 0]
        den[rows] = nd[:, 1]

    labs = labels[vidx]
    hist = np.bincount(labs, minlength=int(labs.max()) + 1)
    cnt = (hist[labs] - 1).astype(np.float64)
    sel = cnt > 0
    n_sel = max(sel.sum(), 1)
    num_v = num[:n_v].astype(np.float64)
    den_v = den[:n_v].astype(np.float64)
    safe_num = np.where(sel, num_v, 1.0)
    safe_den = np.where(sel, den_v, 1.0)
    safe_cnt = np.where(sel, cnt, 1.0)
    lf = (np.log(safe_den) - np.log(safe_num)) / np.log(2.0) + np.log2(safe_cnt)
    total = np.sum(np.where(sel, lf, 0.0)) / n_sel
    return np.float32(total)


def kernel(ent_embeddings, ent_type_ids, ent_mask, W_mu, b_mu, W_sigma, b_sigma):
    from concourse.bass_utils import run_bass_kernel_spmd

    in_maps, meta = _prep(ent_embeddings, ent_type_ids, ent_mask,
                          W_mu, b_mu, W_sigma, b_sigma)
    if in_maps is None:
        return meta
    n_pad = meta[3]
    if n_pad not in _cache:
        _cache[n_pad] = _build(n_pad)
    nc = _cache[n_pad]
    res = run_bass_kernel_spmd(nc, in_maps, list(range(8)))
    return _epilogue(res, meta)
